# revision 62
# baseline (speedup 1.0000x reference)
"""BasisResidualFFN Trainium2 kernel.

Math (per token t):
  recipe_soft = softmax(neuron_recipe, axis=-1)                 [64, 16]
  tr[t, :]    = sum_k w[t,k] * recipe_soft[idx[t,k], :]         [16]
  Y[t, (n,r)] = sum_d x[t,d] * basis_A[n,d,r]
  h[t, r]     = sum_n tr[t,n] * Y[t,(n,r)]
  delta[t, d] = sum_{n,r} basis_A[n,d,r] * tr[t,n] * h[t,r]
  out         = gelu((x + alpha*delta) @ w_up + b_up) @ w_down + b_down

Distribution: pure data parallel. B*S = 4096 tokens sharded 512/core
across 8 NeuronCores; all weights replicated. Everything on device is
computed feature-major (features on partitions, tokens on the free
axis) so no on-device activation transposes are needed; x arrives
pre-transposed from the host and the output is un-transposed there.

Precision: bf16 everywhere except the delta projection, which runs as
fp8 e4m3 DoubleRow matmuls (2x PE throughput): delta = (32*alpha*A2)^T
@ ct with ct in fp8 and the 1/32 compensation riding the PSUM drain
(x is preloaded into PSUM via a 32*identity matmul, so the drain is a
single scaled copy). delta errors enter only through alpha*delta with
alpha ~ 0.1, costing ~1e-4 extra rel err (measured 4e-3 total).

Schedule: the front is chip-wide HBM-bound (all 8 cores pull x/a1/
consts at once) and DMA transfers pay a large per-128-row setup cost,
so the front ships as a handful of fat transfers: one packed const
blob (idxw + C + identities + biases + recipe), x split 4+4 dc across
the sync and scalar rings, a1 and a2 whole. The PE clock ramps over
~3us of continuous busy and re-throttles after idle, so warm-up
matmuls on a memset tile bridge from engine start to x-arrival and a
keep-warm block bridges the xf-drain wait before the up projection;
routing matmuls (recsT, M_i, S^T transposes, RepR) interleave between
YT chunks while Vector runs the scatter. YT and RepH PSUM tiles drain
to bf16 on Scalar so the Vector hadamards run in 2x 16-bit mode. The
wu stream rides the scalar ring, wd the sync ring (prefetches gated
behind a Scalar token so they never steal front bandwidth), and the
output leaves as bf16.
"""

import numpy as np

import concourse.bass as bass
import concourse.mybir as mybir
import concourse.tile as tile
from concourse import bacc
from concourse.bass import ts
from concourse.bass_utils import run_bass_kernel_spmd

P = 128
NCORES = 8
T = 512            # tokens per core
D = 1024
DFF = 4096
NB = 16            # n_basis
R = 32             # rank
NN = 64            # n_neurons
K = 8              # top-k
DC = D // P        # 8 contraction chunks over d
FT = DFF // P      # 32 ff tiles
DT = D // P        # 8 output d tiles
NRT = (NB * R) // P  # 4 (n,r) tiles
TT = T // P        # 4 token tiles per core

# single bf16 const+idxw blob: one DMA transfer (per-128-row DMA setup
# overhead makes many small transfers far slower than one packed one)
BL_IDX, BL_C, BL_ID, BL_ID32, BL_BU, BL_BD, BL_REC, BL_W = (
    0, 64, 192, 320, 448, 480, 488, 504)

F32 = mybir.dt.float32
BF16 = mybir.dt.bfloat16
F8 = mybir.dt.float8e4

DR = mybir.MatmulPerfMode.DoubleRow

NWARM = 26         # keeps the PE busy (and its clock ramped) until x lands
A2S = 32.0         # fp8 scale on alpha*A2; compensated in the xf drain
A1S = 16.0         # fp8 scale on A1; compensated in the C matrix

_BUILT = [None]


def _build_nc():
    nc = bacc.Bacc(None, target_bir_lowering=False)

    # x ships as four DENSE dram tensors: strided dram rows (4KB used of an
    # 8KB stride) halve HBM burst efficiency, and the front is HBM-bound
    xq_d = [nc.dram_tensor(f"xq{q}", [P, 2, T], BF16, kind="ExternalInput")
            for q in range(4)]
    blob_d = nc.dram_tensor("blob1", [P, BL_W], BF16, kind="ExternalInput")
    sel_d = nc.dram_tensor("sel", [NB, NRT, P], BF16, kind="ExternalInput")
    a1_d = nc.dram_tensor("a1", [P, DC, NB * R], F8, kind="ExternalInput")
    a2_d = nc.dram_tensor("a2", [P, 2, 2, DT, P], F8, kind="ExternalInput")
    wu_d = nc.dram_tensor("wu", [FT // 2, P, 2, DC, P], BF16, kind="ExternalInput")
    wd_d = nc.dram_tensor("wd", [DT * 2, P, FT // 2, P], BF16, kind="ExternalInput")
    # output dense per half-tile as well (the last tile's DMA is on the tail)
    out_d = nc.dram_tensor("outT", [DT, 2, P, T // 2], BF16,
                           kind="ExternalOutput")

    AF = mybir.ActivationFunctionType
    ALU = mybir.AluOpType

    with tile.TileContext(nc) as tc:
        with (
            tc.tile_pool(name="const", bufs=1) as constp,
            tc.tile_pool(name="smv", bufs=1) as smv,
            tc.tile_pool(name="small", bufs=2) as small,
            tc.tile_pool(name="stream", bufs=6) as stream,
            tc.tile_pool(name="wdstream", bufs=4) as wdstream,
            tc.tile_pool(name="otp", bufs=2) as otp,
            tc.tile_pool(name="psum", bufs=5, space="PSUM") as psum,
            tc.tile_pool(name="psumA", bufs=2, space="PSUM") as psumA,
            tc.tile_pool(name="psumB", bufs=1, space="PSUM") as psumB,
        ):
            # ---- DMA triggers, ordered by need-time. sync ring: routing
            # consts then x; scalar ring: a1 chunks then a2; gpsimd only
            # memsets (SWDGE is too slow for anything on the path) ----
            warm_sb = constp.tile([P, T], BF16, tag="warm")
            nc.gpsimd.memset(warm_sb[:], 0.0)
            # iota table for the scatter, generated on the (otherwise idle)
            # GpSimd lane instead of DMA'd: repeat(arange(64), 8)
            i512 = constp.tile([P, NN * K], BF16, tag="i512")
            nc.gpsimd.iota(i512[:], pattern=[[1, NN], [0, K]], base=0,
                           channel_multiplier=0,
                           allow_small_or_imprecise_dtypes=True)
            a2 = constp.tile([P, 2, 2, DT, P], F8, tag="a2")

            # one packed blob (idxw + all small consts) and sel on sync,
            # then x dc0-3; a1 then x dc4-7 on scalar. One fat transfer
            # each — per-row DMA setup overhead dominates small transfers.
            blob = constp.tile([P, BL_W], BF16, tag="blob")
            nc.sync.dma_start(blob[:], blob_d[:])
            sel = constp.tile([NB, NRT, P], BF16, tag="sel")
            nc.sync.dma_start(sel[:], sel_d[:])
            xq = [constp.tile([P, 2, T], BF16, tag=f"xq{q}", name=f"xq{q}")
                  for q in range(4)]
            nc.sync.dma_start(xq[0][:], xq_d[0][:])
            nc.sync.dma_start(xq[1][:], xq_d[1][:])

            def xc(dc):
                return xq[dc // 2][:, dc % 2, :]

            a1 = constp.tile([P, DC, NB * R], F8, tag="a1f8")
            nc.scalar.dma_start(a1[:], a1_d[:])
            nc.scalar.dma_start(xq[2][:], xq_d[2][:])
            nc.scalar.dma_start(xq[3][:], xq_d[3][:])
            # a2 rides the scalar ring right behind x (needed only at ~27us;
            # keeping it out of the x window matters — the front is
            # chip-wide HBM-bound with all 8 cores pulling at once)
            nc.scalar.dma_start(a2[:], a2_d[:])
            # exp of the recipe table early (needs only the blob)
            rec = blob[:NN, BL_REC:BL_REC + NB]
            recsb = constp.tile([NN, NB], BF16, tag="recsb")
            ssum = small.tile([NN, 1], F32, tag="ssum")
            nc.scalar.activation(recsb[:], rec, AF.Exp, accum_out=ssum[:])
            # anchor read for the warm-up matmuls (prevents dead-code elim);
            # on Scalar so it cannot delay the Vector scatter or the PE
            warm_anchor = small.tile([P, 1], F32, tag="warm_anchor")

            idxw = blob[:, BL_IDX:BL_IDX + TT * 2 * K].rearrange(
                "p (t k) -> p t k", t=TT)
            # biases back to f32 (scalar/vector bias APs must be f32)
            biasf = constp.tile([P, FT + DT], F32, tag="biasf")
            nc.scalar.activation(biasf[:], blob[:, BL_BU:BL_BD + DT], AF.Copy)
            bu = biasf[:, 0:FT]
            bd = biasf[:, FT:FT + DT]
            cmat = blob[:, BL_C:BL_C + P]
            identb = blob[:, BL_ID:BL_ID + P]
            ident32 = blob[:, BL_ID32:BL_ID32 + P]

            # ---- PE warm-up on the memset tile: starts at user-code time
            # zero with no DMA dependency so the clock ramp begins before the
            # first x/a1 chunk lands ----
            warm_ps = psumB.tile([P, T], F32, tag="b", name="warm")
            for w in range(NWARM):
                nc.tensor.matmul(warm_ps[:], warm_sb[:, :P], warm_sb[:],
                                 start=(w == 0), stop=(w == NWARM - 1))
            nc.scalar.activation(warm_anchor[:], warm_ps[:, 0:1], AF.Copy)

            # pre-issue the first three wu tiles on the scalar queue now, so
            # the up projection's weights are in flight long before the
            # scalar engine reaches the up loop
            wu_tiles = {}
            for ftp in range(3):
                wut = stream.tile([P, 2, DC, P], BF16, tag="wu",
                                  name=f"wu{ftp}")
                nc.scalar.dma_start(wut[:], wu_d[ftp])
                wu_tiles[ftp] = wut

            # ---- routing scatter S[t, neuron] (weighted one-hot): one fused
            # 3-op Vector chain over all four token-tiles ----
            st_sb = constp.tile([NN, T], BF16, tag="st")
            iota4 = i512[:].rearrange(
                "p (o n k) -> p o n k", o=1, k=K).to_broadcast((P, TT, NN, K))
            idx_b = idxw[:, :, 0:K].rearrange(
                "p t (o k) -> p t o k", o=1).to_broadcast((P, TT, NN, K))
            w_b = idxw[:, :, K:2 * K].rearrange(
                "p t (o k) -> p t o k", o=1).to_broadcast((P, TT, NN, K))
            sk = smv.tile([P, TT, NN, K], BF16, tag="sk")
            s_all = smv.tile([P, TT, NN], BF16, tag="s")
            nc.vector.tensor_tensor(sk[:], iota4, idx_b, ALU.is_equal)
            nc.vector.tensor_tensor(sk[:], sk[:], w_b, ALU.mult)
            # pairwise-tree reduction over k (faster than reduce_sum on DVE)
            nc.vector.tensor_tensor(sk[:, :, :, 0:4], sk[:, :, :, 0:4],
                                    sk[:, :, :, 4:8], ALU.add)
            nc.vector.tensor_tensor(sk[:, :, :, 0:2], sk[:, :, :, 0:2],
                                    sk[:, :, :, 2:4], ALU.add)
            nc.vector.tensor_tensor(
                s_all[:].rearrange("p t (n o) -> p t n o", o=1),
                sk[:, :, :, 0:1], sk[:, :, :, 1:2], ALU.add)
            rsum = small.tile([NN, 1], F32, tag="rsum")
            nc.vector.reciprocal(rsum[:], ssum[:])

            yt_ps = [psum.tile([P, T], F32, tag="ps", name=f"yt{i}")
                     for i in range(NRT)]

            def yt_block(i):
                for dc in range(DC):
                    nc.tensor.matmul(yt_ps[i][:], a1[:, dc, ts(i, P)], xc(dc),
                                     start=(dc == 0), stop=(dc == DC - 1))

            # ---- routing matmuls run first: rT/M_i need only recsb/sel
            # (landed long before x), stp right at x-arrival (the scatter is
            # done by then), so RepR completes during yt block 0 and the
            # per-tile drain/wyt/rh pipeline can chase each YT block ----
            rT_ps = psumB.tile([NB, NN], BF16, tag="b", name="rTps")
            nc.tensor.transpose(rT_ps[:], recsb[:], identb[:NN, :NN])
            recsT = constp.tile([NB, NN], BF16, tag="recsT")
            nc.scalar.activation(recsT[:], rT_ps[:], AF.Copy)

            m_sb = []
            for i in range(NRT):
                mp = psumA.tile([NN, P], F32, tag="rp", name=f"m{i}")
                nc.tensor.matmul(mp[:], recsT[:], sel[:, i, :],
                                 start=True, stop=True)
                ms = constp.tile([NN, P], BF16, tag=f"m{i}", name=f"ms{i}")
                nc.scalar.activation(ms[:], mp[:], AF.Copy)
                m_sb.append(ms)

            # routing transposes: all four into one PSUM tile, one drain
            stp_all = psumB.tile([NN, TT * P], BF16, tag="b", name="stp")
            for tt in range(TT):
                nc.tensor.transpose(stp_all[:, ts(tt, P)], s_all[:, tt, :],
                                    identb)
            nc.scalar.activation(st_sb[:], stp_all[:], AF.Copy,
                                 scale=rsum[:, 0:1])

            yt_block(0)

            # RepR[i][(n,r), t] = tr[t, n(i,p)] = M_i^T @ S^T, into one tile
            reprall = constp.tile([P, NRT, T], BF16, tag="reprall")
            rr_ps = [psumA.tile([P, T], F32, tag="rp", name=f"rp{i}")
                     for i in range(NRT)]
            for i in range(NRT):
                nc.tensor.matmul(rr_ps[i][:], m_sb[i][:], st_sb[:],
                                 start=True, stop=True)
                if i % 2 == 0:
                    nc.vector.tensor_copy(reprall[:, i, :], rr_ps[i][:])
                else:
                    nc.scalar.activation(reprall[:, i, :], rr_ps[i][:], AF.Copy)

            # prefetch the first two wd tiles on the (now idle) sync ring,
            # each gated by a one-element Scalar copy so the 2MB cannot
            # steal ring bandwidth from the front stream
            wd_tiles = {}
            for dt in range(2):
                for h in range(2):
                    wdt = wdstream.tile([P, FT // 2, P], BF16, tag="wd",
                                        name=f"wd{dt}_{h}")
                    nc.scalar.activation(wdt[:, 0, 0:1], warm_anchor[:, 0:1],
                                         AF.Copy)
                    nc.sync.dma_start(wdt[:], wd_d[dt * 2 + h])
                    wd_tiles[(dt, h)] = wdt

            # ---- i-outer YT with the chain chasing: WYT = YT * RepR,
            # RepH = C^T @ sum-over-i WYT (C = qred @ trep host-fused). YT
            # drains to bf16 on Scalar so the DVE hadamards run in 2x
            # 16-bit mode; tiles 0-2 drain/multiply while later YT blocks
            # still run, leaving only tile 3's chain serial ----
            yt_sb = constp.tile([P, NRT, T], BF16, tag="ytsb")
            wyt = constp.tile([P, NRT, T], BF16, tag="wyt")
            xf = constp.tile([P, DC, T], BF16, tag="a1f8", name="xf")
            rh_ps = psumA.tile([P, T], F32, tag="rp", name="rh")
            dl_ps = {}

            # Scalar drains + Vector hadamards chase each YT block; the rh
            # matmuls are emitted only after the last block so the in-order
            # PE never stalls on the Vector queue mid-YT
            def chain_sv(i):
                nc.scalar.activation(yt_sb[:, i, :], yt_ps[i][:], AF.Copy)
                nc.vector.tensor_mul(out=wyt[:, i, :], in0=yt_sb[:, i, :],
                                     in1=reprall[:, i, :])

            yt_block(1)
            chain_sv(0)
            yt_block(2)
            chain_sv(1)
            yt_block(3)
            chain_sv(2)
            chain_sv(3)
            for i in range(NRT):
                nc.tensor.matmul(rh_ps[:], cmat, wyt[:, i, :],
                                 start=(i == 0), stop=(i == NRT - 1))
                dl_ps[i] = psum.tile([P, T], F32, tag="ps", name=f"dl{i}")
                nc.tensor.matmul(dl_ps[i][:], ident32, xc(i),
                                 start=True, stop=False)
            dl_ps[4] = psum.tile([P, T], F32, tag="ps", name="dl4")
            nc.tensor.matmul(dl_ps[4][:], ident32, xc(4),
                             start=True, stop=False)

            # CT = RepH * RepR in fp8; rh drains to bf16 first (Scalar) so
            # the ct multiplies also run at 2x on the DVE
            rh_sb = constp.tile([P, T], BF16, tag="rhsb")
            nc.scalar.activation(rh_sb[:], rh_ps[:], AF.Copy)
            ct = constp.tile([P, NRT, T], F8, tag="ct")
            for i in range(NRT):
                nc.vector.tensor_tensor(ct[:, i, :], rh_sb[:],
                                        reprall[:, i, :], ALU.mult)

            # ---- deltaT: fp8 DoubleRow, dt-outer so drains chase;  xf =
            # (32*x + 32*alpha*delta) / 32 on alternating Scalar/Vector ----
            def drain_xf(dt):
                if dt % 2 == 0:
                    nc.scalar.activation(xf[:, dt, :], dl_ps[dt][:], AF.Copy,
                                         scale=1.0 / A2S)
                else:
                    nc.vector.tensor_scalar_mul(xf[:, dt, :], dl_ps[dt][:],
                                                1.0 / A2S)

            def delta_block(dt):
                for pi in range(2):
                    nc.tensor.matmul(dl_ps[dt][:], a2[:, pi, :, dt, :],
                                     ct[:, 2 * pi:2 * pi + 2, :],
                                     start=False, stop=(pi == 1),
                                     perf_mode=DR)
                drain_xf(dt)

            for dt in range(3):
                delta_block(dt)
            for dt in range(5, DT):
                dl_ps[dt] = psum.tile([P, T], F32, tag="ps", name=f"dl{dt}")
                nc.tensor.matmul(dl_ps[dt][:], ident32, xc(dt),
                                 start=True, stop=False)
                delta_block(dt - 2)
            delta_block(6)
            delta_block(7)

            # keep-warm matmuls: cover the ct-pacing wait so the clock stays
            # up going into the 55us up-projection
            keep_ps = psumB.tile([P, T], F32, tag="b", name="keep")
            for w in range(4):
                nc.tensor.matmul(keep_ps[:], warm_sb[:, :P], warm_sb[:],
                                 start=(w == 0), stop=(w == 3))
            keep_anchor = small.tile([P, 1], F32, tag="warm_anchor",
                                     name="keep_anchor")
            nc.scalar.activation(keep_anchor[:], keep_ps[:, 0:1], AF.Copy)

            # ---- FFN up + exact gelu; wu streams on the scalar ring so it
            # never competes with x/consts on the sync ring. The first wu
            # pair runs dc-outer so the PE consumes each xf chunk as its
            # drain lands instead of stalling for all eight ----
            g = constp.tile([P, FT, T], BF16, tag="g")
            for ftp in range(FT // 2):
                if ftp in wu_tiles:
                    wu = wu_tiles.pop(ftp)
                else:
                    wu = stream.tile([P, 2, DC, P], BF16, tag="wu",
                                     name=f"wu{ftp}")
                    nc.scalar.dma_start(wu[:], wu_d[ftp])
                if ftp == 0:
                    # psumA slots (m/rr/rh) free right after ct, well before
                    # the psum-pool dl slots do — no wait on the drains
                    u_pair = [psumA.tile([P, T], F32, tag="rp", name=f"u{j}")
                              for j in range(2)]
                    for dc in range(DC):
                        for j in range(2):
                            nc.tensor.matmul(u_pair[j][:], wu[:, j, dc, :],
                                             xf[:, dc, :], start=(dc == 0),
                                             stop=(dc == DC - 1))
                    for j in range(2):
                        nc.scalar.activation(g[:, j, :], u_pair[j][:], AF.Gelu,
                                             bias=bu[:, j:j + 1], scale=1.0)
                    continue
                for j in range(2):
                    ft = 2 * ftp + j
                    u_ps = psum.tile([P, T], F32, tag="ps", name=f"u{ft}")
                    for dc in range(DC):
                        nc.tensor.matmul(u_ps[:], wu[:, j, dc, :], xf[:, dc, :],
                                         start=(dc == 0), stop=(dc == DC - 1))
                    nc.scalar.activation(g[:, ft, :], u_ps[:], AF.Gelu,
                                         bias=bu[:, ft:ft + 1], scale=1.0)

            # ---- FFN down + bias; bias-add split Vector/Scalar and the
            # output DMA split across two queues to shorten the tail ----
            TH = T // 2
            for dt in range(DT):
                o_ps = psum.tile([P, T], F32, tag="ps", name=f"o{dt}")
                for h in range(2):
                    wd = wd_tiles.pop((dt, h))
                    for fc in range(FT // 2):
                        fcg = h * (FT // 2) + fc
                        nc.tensor.matmul(o_ps[:], wd[:, fc, :], g[:, fcg, :],
                                         start=(fcg == 0), stop=(fcg == FT - 1))
                    if dt + 2 < DT:
                        wdt = wdstream.tile([P, FT // 2, P], BF16, tag="wd",
                                            name=f"wd{dt + 2}_{h}")
                        nc.sync.dma_start(wdt[:], wd_d[(dt + 2) * 2 + h])
                        wd_tiles[(dt + 2, h)] = wdt
                otv = otp.tile([P, TH], BF16, tag="otv", name=f"otv{dt}")
                nc.vector.tensor_scalar_add(otv[:], o_ps[:, 0:TH],
                                            bd[:, dt:dt + 1])
                nc.sync.dma_start(out_d[dt, 0], otv[:])
                otg = otp.tile([P, TH], BF16, tag="otg", name=f"otg{dt}")
                nc.scalar.activation(otg[:], o_ps[:, TH:T], AF.Identity,
                                     bias=bd[:, dt:dt + 1], scale=1.0)
                nc.scalar.dma_start(out_d[dt, 1], otg[:])

    nc.finalize()
    return nc


def _get_nc():
    if _BUILT[0] is None:
        _BUILT[0] = _build_nc()
    return _BUILT[0]


def kernel(x, neuron_idx, neuron_weights, neuron_recipe, basis_A,
           w_up_w, w_up_b, w_down_w, w_down_b, alpha):
    import ml_dtypes
    nc = _get_nc()

    x = np.asarray(x, dtype=np.float32).reshape(NCORES * T, D)
    idxf = np.asarray(neuron_idx).astype(np.float32).reshape(NCORES * T, K)
    wgt = np.asarray(neuron_weights, dtype=np.float32).reshape(NCORES * T, K)
    rec = np.asarray(neuron_recipe, dtype=np.float32)
    bA = np.asarray(basis_A, dtype=np.float32)
    wu = np.asarray(w_up_w, dtype=np.float32)
    bu_in = np.asarray(w_up_b, dtype=np.float32)
    wd = np.asarray(w_down_w, dtype=np.float32)
    bd_in = np.asarray(w_down_b, dtype=np.float32)
    alpha_f = float(np.asarray(alpha, dtype=np.float32))

    # replicated operands, packed into the on-device layouts
    # A1 in fp8, scaled by 16 to stay in e4m3 normal range (1/16 in C)
    a1 = np.ascontiguousarray(
        np.clip(bA.transpose(1, 0, 2).reshape(D, NB * R) * A1S, -240.0, 240.0)
        .reshape(DC, P, NB * R).transpose(1, 0, 2)
    ).astype(ml_dtypes.float8_e4m3)
    # delta projection in fp8: 32*alpha*A2, [(pair, j, p), (dt, c)] packed
    a2m = np.clip(bA.transpose(0, 2, 1).reshape(NB * R, D) * (alpha_f * A2S),
                  -240.0, 240.0)
    a2 = np.ascontiguousarray(
        a2m.reshape(2, 2, P, DT, P).transpose(2, 0, 1, 3, 4)
    ).astype(ml_dtypes.float8_e4m3)
    wu_p = np.ascontiguousarray(
        wu.reshape(DC, P, FT // 2, 2, P).transpose(2, 1, 3, 0, 4)
    ).astype(ml_dtypes.bfloat16)
    wd_p = np.ascontiguousarray(
        wd.reshape(2, FT // 2, P, DT, P).transpose(3, 0, 2, 1, 4)
        .reshape(DT * 2, P, FT // 2, P)).astype(ml_dtypes.bfloat16)

    # packed per-core blob: idxw + C + identities + biases + recipe, bf16
    blobc = np.zeros((P, BL_W), dtype=np.float32)
    # C = qred @ trep fused: C[q, p] = 1/A1S iff q % R == p % R
    blobc[:, BL_C:BL_C + P] = (
        np.arange(P)[:, None] % R == np.arange(P)[None, :] % R) / A1S
    blobc[:, BL_ID:BL_ID + P] = np.eye(P, dtype=np.float32)
    blobc[:, BL_ID32:BL_ID32 + P] = np.eye(P, dtype=np.float32) * A2S
    blobc[:, BL_BU:BL_BU + FT] = bu_in.reshape(FT, P).T
    blobc[:, BL_BD:BL_BD + DT] = bd_in.reshape(DT, P).T
    blobc[:NN, BL_REC:BL_REC + NB] = rec

    # SEL[n, i, m] = 1 iff n in [4i, 4i+4) and m // 32 == n - 4i
    sel = np.zeros((NB, NRT, P), dtype=np.float32)
    for n in range(NB):
        i, nloc = divmod(n, NRT)
        sel[n, i, nloc * R:(nloc + 1) * R] = 1.0
    sel = sel.astype(ml_dtypes.bfloat16)

    shared = {
        "sel": sel, "a1": a1, "a2": a2, "wu": wu_p, "wd": wd_p,
    }
    in_maps = []
    idxw = np.concatenate([idxf, wgt], axis=1).astype(
        ml_dtypes.bfloat16)  # [N*T, 16]
    for c in range(NCORES):
        xc = x[c * T:(c + 1) * T]  # [T, D]
        xtc = np.ascontiguousarray(xc.T.reshape(DC, P, T).transpose(1, 0, 2))
        xtbc = xtc.astype(ml_dtypes.bfloat16)
        xqs = {f"xq{q}": np.ascontiguousarray(xtbc[:, 2 * q:2 * q + 2, :])
               for q in range(4)}
        iwc = np.ascontiguousarray(
            idxw[c * T:(c + 1) * T].reshape(TT, P, 2 * K).transpose(1, 0, 2))
        blob_c = blobc.astype(ml_dtypes.bfloat16)
        blob_c[:, BL_IDX:BL_IDX + TT * 2 * K] = iwc.reshape(P, TT * 2 * K)
        in_maps.append({**xqs, "blob1": blob_c, **shared})

    res = run_bass_kernel_spmd(nc, in_maps, core_ids=list(range(NCORES)))

    out = np.empty((NCORES * T, D), dtype=np.float32)
    for c in range(NCORES):
        oq = res.results[c]["outT"].astype(np.float32)  # [DT, 2, P, TH]
        ot = oq.transpose(2, 0, 1, 3).reshape(P, DT, T)
        out[c * T:(c + 1) * T] = ot.transpose(1, 0, 2).reshape(D, T).T
    return out.reshape(2, 2048, D)


# revision 64
# speedup vs baseline: 1.0096x; 1.0096x over previous
"""BasisResidualFFN Trainium2 kernel.

Math (per token t):
  recipe_soft = softmax(neuron_recipe, axis=-1)                 [64, 16]
  tr[t, :]    = sum_k w[t,k] * recipe_soft[idx[t,k], :]         [16]
  Y[t, (n,r)] = sum_d x[t,d] * basis_A[n,d,r]
  h[t, r]     = sum_n tr[t,n] * Y[t,(n,r)]
  delta[t, d] = sum_{n,r} basis_A[n,d,r] * tr[t,n] * h[t,r]
  out         = gelu((x + alpha*delta) @ w_up + b_up) @ w_down + b_down

Distribution: pure data parallel. B*S = 4096 tokens sharded 512/core
across 8 NeuronCores; all weights replicated. Everything on device is
computed feature-major (features on partitions, tokens on the free
axis) so no on-device activation transposes are needed; x arrives
pre-transposed from the host and the output is un-transposed there.

Precision: bf16 everywhere except the delta projection, which runs as
fp8 e4m3 DoubleRow matmuls (2x PE throughput): delta = (32*alpha*A2)^T
@ ct with ct in fp8 and the 1/32 compensation riding the PSUM drain
(x is preloaded into PSUM via a 32*identity matmul, so the drain is a
single scaled copy). delta errors enter only through alpha*delta with
alpha ~ 0.1, costing ~1e-4 extra rel err (measured 4e-3 total).

Schedule: the front is chip-wide HBM-bound (all 8 cores pull x/a1/
consts at once) and DMA transfers pay a large per-128-row setup cost,
so the front ships as a handful of fat transfers: one packed const
blob (idxw + C + identities + biases + recipe), x split 4+4 dc across
the sync and scalar rings, a1 and a2 whole. The PE clock ramps over
~3us of continuous busy and re-throttles after idle, so warm-up
matmuls on a memset tile bridge from engine start to x-arrival and a
keep-warm block bridges the xf-drain wait before the up projection;
routing matmuls (recsT, M_i, S^T transposes, RepR) interleave between
YT chunks while Vector runs the scatter. YT and RepH PSUM tiles drain
to bf16 on Scalar so the Vector hadamards run in 2x 16-bit mode. The
wu stream rides the scalar ring, wd the sync ring (prefetches gated
behind a Scalar token so they never steal front bandwidth), and the
output leaves as bf16.
"""

import numpy as np

import concourse.bass as bass
import concourse.mybir as mybir
import concourse.tile as tile
from concourse import bacc
from concourse.bass import ts
from concourse.bass_utils import run_bass_kernel_spmd

P = 128
NCORES = 8
T = 512            # tokens per core
D = 1024
DFF = 4096
NB = 16            # n_basis
R = 32             # rank
NN = 64            # n_neurons
K = 8              # top-k
DC = D // P        # 8 contraction chunks over d
FT = DFF // P      # 32 ff tiles
DT = D // P        # 8 output d tiles
NRT = (NB * R) // P  # 4 (n,r) tiles
TT = T // P        # 4 token tiles per core

# single bf16 const+idxw blob: one DMA transfer (per-128-row DMA setup
# overhead makes many small transfers far slower than one packed one)
BL_IDX, BL_C, BL_ID, BL_ID32, BL_BU, BL_BD, BL_REC, BL_W = (
    0, 64, 192, 320, 448, 480, 488, 504)

F32 = mybir.dt.float32
BF16 = mybir.dt.bfloat16
F8 = mybir.dt.float8e4

DR = mybir.MatmulPerfMode.DoubleRow

NWARM = 26         # keeps the PE busy (and its clock ramped) until x lands
A2S = 32.0         # fp8 scale on alpha*A2; compensated in the xf drain
A1S = 16.0         # fp8 scale on A1; compensated in the C matrix

_BUILT = [None]


def _build_nc():
    nc = bacc.Bacc(None, target_bir_lowering=False)

    # x ships as four DENSE dram tensors: strided dram rows (4KB used of an
    # 8KB stride) halve HBM burst efficiency, and the front is HBM-bound
    xq_d = [nc.dram_tensor(f"xq{q}", [P, 2, T], BF16, kind="ExternalInput")
            for q in range(4)]
    blob_d = nc.dram_tensor("blob1", [P, BL_W], BF16, kind="ExternalInput")
    sel_d = nc.dram_tensor("sel", [NB, NRT, P], BF16, kind="ExternalInput")
    a1_d = nc.dram_tensor("a1", [P, DC, NB * R], F8, kind="ExternalInput")
    a2_d = nc.dram_tensor("a2", [P, 2, 2, DT, P], F8, kind="ExternalInput")
    wu_d = nc.dram_tensor("wu", [FT // 2, P, 2, DC, P], BF16, kind="ExternalInput")
    wd_d = nc.dram_tensor("wd", [DT * 2, P, FT // 2, P], BF16, kind="ExternalInput")
    # output dense per half-tile as well (the last tile's DMA is on the tail)
    out_d = nc.dram_tensor("outT", [DT, 2, P, T // 2], BF16,
                           kind="ExternalOutput")

    AF = mybir.ActivationFunctionType
    ALU = mybir.AluOpType

    with tile.TileContext(nc) as tc:
        with (
            tc.tile_pool(name="const", bufs=1) as constp,
            tc.tile_pool(name="smv", bufs=1) as smv,
            tc.tile_pool(name="small", bufs=2) as small,
            tc.tile_pool(name="stream", bufs=6) as stream,
            tc.tile_pool(name="wdstream", bufs=4) as wdstream,
            tc.tile_pool(name="otp", bufs=2) as otp,
            tc.tile_pool(name="psum", bufs=5, space="PSUM") as psum,
            tc.tile_pool(name="psumA", bufs=2, space="PSUM") as psumA,
            tc.tile_pool(name="psumB", bufs=1, space="PSUM") as psumB,
        ):
            # ---- DMA triggers, ordered by need-time. sync ring: routing
            # consts then x; scalar ring: a1 chunks then a2; gpsimd only
            # memsets (SWDGE is too slow for anything on the path) ----
            warm_sb = constp.tile([P, T], BF16, tag="warm")
            nc.gpsimd.memset(warm_sb[:], 0.0)
            # iota table for the scatter, generated on the (otherwise idle)
            # GpSimd lane instead of DMA'd: repeat(arange(64), 8)
            i512 = constp.tile([P, NN * K], BF16, tag="i512")
            nc.gpsimd.iota(i512[:], pattern=[[1, NN], [0, K]], base=0,
                           channel_multiplier=0,
                           allow_small_or_imprecise_dtypes=True)
            a2 = constp.tile([P, 2, 2, DT, P], F8, tag="a2")

            # one packed blob (idxw + all small consts) and sel on sync,
            # then x dc0-3; a1 then x dc4-7 on scalar. One fat transfer
            # each — per-row DMA setup overhead dominates small transfers.
            blob = constp.tile([P, BL_W], BF16, tag="blob")
            nc.sync.dma_start(blob[:], blob_d[:])
            sel = constp.tile([NB, NRT, P], BF16, tag="sel")
            nc.sync.dma_start(sel[:], sel_d[:])
            xq = [constp.tile([P, 2, T], BF16, tag=f"xq{q}", name=f"xq{q}")
                  for q in range(4)]
            nc.sync.dma_start(xq[0][:], xq_d[0][:])
            nc.sync.dma_start(xq[1][:], xq_d[1][:])

            def xc(dc):
                return xq[dc // 2][:, dc % 2, :]

            a1 = constp.tile([P, DC, NB * R], F8, tag="a1f8")
            nc.scalar.dma_start(a1[:], a1_d[:])
            nc.scalar.dma_start(xq[2][:], xq_d[2][:])
            # the last x quarter rides the otherwise-idle SWDGE ring: slower,
            # but it adds real parallel HBM bandwidth to the chip-wide-bound
            # front, and dc6-7 aren't consumed until ~13us after it lands
            nc.gpsimd.dma_start(xq[3][:], xq_d[3][:])
            # exp of the recipe table early (needs only the blob)
            rec = blob[:NN, BL_REC:BL_REC + NB]
            recsb = constp.tile([NN, NB], BF16, tag="recsb")
            ssum = small.tile([NN, 1], F32, tag="ssum")
            nc.scalar.activation(recsb[:], rec, AF.Exp, accum_out=ssum[:])
            # anchor read for the warm-up matmuls (prevents dead-code elim);
            # on Scalar so it cannot delay the Vector scatter or the PE
            warm_anchor = small.tile([P, 1], F32, tag="warm_anchor")

            idxw = blob[:, BL_IDX:BL_IDX + TT * 2 * K].rearrange(
                "p (t k) -> p t k", t=TT)
            # biases back to f32 (scalar/vector bias APs must be f32)
            biasf = constp.tile([P, FT + DT], F32, tag="biasf")
            nc.scalar.activation(biasf[:], blob[:, BL_BU:BL_BD + DT], AF.Copy)
            bu = biasf[:, 0:FT]
            bd = biasf[:, FT:FT + DT]
            cmat = blob[:, BL_C:BL_C + P]
            identb = blob[:, BL_ID:BL_ID + P]
            ident32 = blob[:, BL_ID32:BL_ID32 + P]

            # ---- PE warm-up on the memset tile: starts at user-code time
            # zero with no DMA dependency so the clock ramp begins before the
            # first x/a1 chunk lands ----
            warm_ps = psumB.tile([P, T], F32, tag="b", name="warm")
            for w in range(NWARM):
                nc.tensor.matmul(warm_ps[:], warm_sb[:, :P], warm_sb[:],
                                 start=(w == 0), stop=(w == NWARM - 1))
            nc.scalar.activation(warm_anchor[:], warm_ps[:, 0:1], AF.Copy)

            # pre-issue the first three wu tiles on the scalar queue now, so
            # the up projection's weights are in flight long before the
            # scalar engine reaches the up loop
            wu_tiles = {}
            for ftp in range(3):
                wut = stream.tile([P, 2, DC, P], BF16, tag="wu",
                                  name=f"wu{ftp}")
                nc.scalar.dma_start(wut[:], wu_d[ftp])
                wu_tiles[ftp] = wut
                if ftp == 0:
                    # a2 behind wu0 (needed only at ~25us): keeps it out of
                    # the x window on the chip-wide HBM-bound front
                    nc.scalar.dma_start(a2[:], a2_d[:])

            # ---- routing scatter S[t, neuron] (weighted one-hot): one fused
            # 3-op Vector chain over all four token-tiles ----
            st_sb = constp.tile([NN, T], BF16, tag="st")
            iota4 = i512[:].rearrange(
                "p (o n k) -> p o n k", o=1, k=K).to_broadcast((P, TT, NN, K))
            idx_b = idxw[:, :, 0:K].rearrange(
                "p t (o k) -> p t o k", o=1).to_broadcast((P, TT, NN, K))
            w_b = idxw[:, :, K:2 * K].rearrange(
                "p t (o k) -> p t o k", o=1).to_broadcast((P, TT, NN, K))
            sk = smv.tile([P, TT, NN, K], BF16, tag="sk")
            s_all = smv.tile([P, TT, NN], BF16, tag="s")
            nc.vector.tensor_tensor(sk[:], iota4, idx_b, ALU.is_equal)
            nc.vector.tensor_tensor(sk[:], sk[:], w_b, ALU.mult)
            # pairwise-tree reduction over k (faster than reduce_sum on DVE)
            nc.vector.tensor_tensor(sk[:, :, :, 0:4], sk[:, :, :, 0:4],
                                    sk[:, :, :, 4:8], ALU.add)
            nc.vector.tensor_tensor(sk[:, :, :, 0:2], sk[:, :, :, 0:2],
                                    sk[:, :, :, 2:4], ALU.add)
            nc.vector.tensor_tensor(
                s_all[:].rearrange("p t (n o) -> p t n o", o=1),
                sk[:, :, :, 0:1], sk[:, :, :, 1:2], ALU.add)
            rsum = small.tile([NN, 1], F32, tag="rsum")
            nc.vector.reciprocal(rsum[:], ssum[:])

            yt_ps = [psum.tile([P, T], F32, tag="ps", name=f"yt{i}")
                     for i in range(NRT)]

            def yt_block(i):
                for dc in range(DC):
                    nc.tensor.matmul(yt_ps[i][:], a1[:, dc, ts(i, P)], xc(dc),
                                     start=(dc == 0), stop=(dc == DC - 1))

            # ---- routing matmuls run first: rT/M_i need only recsb/sel
            # (landed long before x), stp right at x-arrival (the scatter is
            # done by then), so RepR completes during yt block 0 and the
            # per-tile drain/wyt/rh pipeline can chase each YT block ----
            rT_ps = psumB.tile([NB, NN], BF16, tag="b", name="rTps")
            nc.tensor.transpose(rT_ps[:], recsb[:], identb[:NN, :NN])
            recsT = constp.tile([NB, NN], BF16, tag="recsT")
            nc.scalar.activation(recsT[:], rT_ps[:], AF.Copy)

            m_sb = []
            for i in range(NRT):
                mp = psumA.tile([NN, P], F32, tag="rp", name=f"m{i}")
                nc.tensor.matmul(mp[:], recsT[:], sel[:, i, :],
                                 start=True, stop=True)
                ms = constp.tile([NN, P], BF16, tag=f"m{i}", name=f"ms{i}")
                nc.scalar.activation(ms[:], mp[:], AF.Copy)
                m_sb.append(ms)

            # routing transposes: all four into one PSUM tile, one drain
            stp_all = psumB.tile([NN, TT * P], BF16, tag="b", name="stp")
            for tt in range(TT):
                nc.tensor.transpose(stp_all[:, ts(tt, P)], s_all[:, tt, :],
                                    identb)
            nc.scalar.activation(st_sb[:], stp_all[:], AF.Copy,
                                 scale=rsum[:, 0:1])

            yt_block(0)

            # RepR[i][(n,r), t] = tr[t, n(i,p)] = M_i^T @ S^T, into one tile
            reprall = constp.tile([P, NRT, T], BF16, tag="reprall")
            rr_ps = [psumA.tile([P, T], F32, tag="rp", name=f"rp{i}")
                     for i in range(NRT)]
            for i in range(NRT):
                nc.tensor.matmul(rr_ps[i][:], m_sb[i][:], st_sb[:],
                                 start=True, stop=True)
                if i % 2 == 0:
                    nc.vector.tensor_copy(reprall[:, i, :], rr_ps[i][:])
                else:
                    nc.scalar.activation(reprall[:, i, :], rr_ps[i][:], AF.Copy)

            # prefetch the first two wd tiles on the (now idle) sync ring,
            # each gated by a one-element Scalar copy so the 2MB cannot
            # steal ring bandwidth from the front stream
            wd_tiles = {}
            for dt in range(2):
                for h in range(2):
                    wdt = wdstream.tile([P, FT // 2, P], BF16, tag="wd",
                                        name=f"wd{dt}_{h}")
                    nc.scalar.activation(wdt[:, 0, 0:1], warm_anchor[:, 0:1],
                                         AF.Copy)
                    nc.sync.dma_start(wdt[:], wd_d[dt * 2 + h])
                    wd_tiles[(dt, h)] = wdt

            # ---- i-outer YT with the chain chasing: WYT = YT * RepR,
            # RepH = C^T @ sum-over-i WYT (C = qred @ trep host-fused). YT
            # drains to bf16 on Scalar so the DVE hadamards run in 2x
            # 16-bit mode; tiles 0-2 drain/multiply while later YT blocks
            # still run, leaving only tile 3's chain serial ----
            yt_sb = constp.tile([P, NRT, T], BF16, tag="ytsb")
            wyt = constp.tile([P, NRT, T], BF16, tag="wyt")
            xf = constp.tile([P, DC, T], BF16, tag="a1f8", name="xf")
            rh_ps = psumA.tile([P, T], F32, tag="rp", name="rh")
            dl_ps = {}

            # Scalar drains + Vector hadamards chase each YT block; the rh
            # matmuls are emitted only after the last block so the in-order
            # PE never stalls on the Vector queue mid-YT
            def chain_sv(i):
                nc.scalar.activation(yt_sb[:, i, :], yt_ps[i][:], AF.Copy)
                nc.vector.tensor_mul(out=wyt[:, i, :], in0=yt_sb[:, i, :],
                                     in1=reprall[:, i, :])

            yt_block(1)
            chain_sv(0)
            yt_block(2)
            chain_sv(1)
            yt_block(3)
            chain_sv(2)
            chain_sv(3)
            for i in range(NRT):
                nc.tensor.matmul(rh_ps[:], cmat, wyt[:, i, :],
                                 start=(i == 0), stop=(i == NRT - 1))
                dl_ps[i] = psum.tile([P, T], F32, tag="ps", name=f"dl{i}")
                nc.tensor.matmul(dl_ps[i][:], ident32, xc(i),
                                 start=True, stop=False)
            dl_ps[4] = psum.tile([P, T], F32, tag="ps", name="dl4")
            nc.tensor.matmul(dl_ps[4][:], ident32, xc(4),
                             start=True, stop=False)

            # CT = RepH * RepR in fp8; rh drains to bf16 first (Scalar) so
            # the ct multiplies also run at 2x on the DVE
            rh_sb = constp.tile([P, T], BF16, tag="rhsb")
            nc.scalar.activation(rh_sb[:], rh_ps[:], AF.Copy)
            ct = constp.tile([P, NRT, T], F8, tag="ct")
            for i in range(NRT):
                nc.vector.tensor_tensor(ct[:, i, :], rh_sb[:],
                                        reprall[:, i, :], ALU.mult)

            # ---- deltaT: fp8 DoubleRow, dt-outer so drains chase;  xf =
            # (32*x + 32*alpha*delta) / 32 on alternating Scalar/Vector ----
            def drain_xf(dt):
                if dt % 2 == 0:
                    nc.scalar.activation(xf[:, dt, :], dl_ps[dt][:], AF.Copy,
                                         scale=1.0 / A2S)
                else:
                    nc.vector.tensor_scalar_mul(xf[:, dt, :], dl_ps[dt][:],
                                                1.0 / A2S)

            def delta_block(dt):
                for pi in range(2):
                    nc.tensor.matmul(dl_ps[dt][:], a2[:, pi, :, dt, :],
                                     ct[:, 2 * pi:2 * pi + 2, :],
                                     start=False, stop=(pi == 1),
                                     perf_mode=DR)
                drain_xf(dt)

            for dt in range(3):
                delta_block(dt)
            for dt in range(5, DT):
                dl_ps[dt] = psum.tile([P, T], F32, tag="ps", name=f"dl{dt}")
                nc.tensor.matmul(dl_ps[dt][:], ident32, xc(dt),
                                 start=True, stop=False)
                delta_block(dt - 2)
            delta_block(6)
            delta_block(7)

            # keep-warm matmuls: cover the ct-pacing wait so the clock stays
            # up going into the 55us up-projection
            keep_ps = psumB.tile([P, T], F32, tag="b", name="keep")
            for w in range(4):
                nc.tensor.matmul(keep_ps[:], warm_sb[:, :P], warm_sb[:],
                                 start=(w == 0), stop=(w == 3))
            keep_anchor = small.tile([P, 1], F32, tag="warm_anchor",
                                     name="keep_anchor")
            nc.scalar.activation(keep_anchor[:], keep_ps[:, 0:1], AF.Copy)

            # ---- FFN up + exact gelu; wu streams on the scalar ring so it
            # never competes with x/consts on the sync ring. The first wu
            # pair runs dc-outer so the PE consumes each xf chunk as its
            # drain lands instead of stalling for all eight ----
            g = constp.tile([P, FT, T], BF16, tag="g")
            for ftp in range(FT // 2):
                if ftp in wu_tiles:
                    wu = wu_tiles.pop(ftp)
                else:
                    wu = stream.tile([P, 2, DC, P], BF16, tag="wu",
                                     name=f"wu{ftp}")
                    nc.scalar.dma_start(wu[:], wu_d[ftp])
                if ftp == 0:
                    # psumA slots (m/rr/rh) free right after ct, well before
                    # the psum-pool dl slots do — no wait on the drains
                    u_pair = [psumA.tile([P, T], F32, tag="rp", name=f"u{j}")
                              for j in range(2)]
                    for dc in range(DC):
                        for j in range(2):
                            nc.tensor.matmul(u_pair[j][:], wu[:, j, dc, :],
                                             xf[:, dc, :], start=(dc == 0),
                                             stop=(dc == DC - 1))
                    for j in range(2):
                        nc.scalar.activation(g[:, j, :], u_pair[j][:], AF.Gelu,
                                             bias=bu[:, j:j + 1], scale=1.0)
                    continue
                for j in range(2):
                    ft = 2 * ftp + j
                    u_ps = psum.tile([P, T], F32, tag="ps", name=f"u{ft}")
                    for dc in range(DC):
                        nc.tensor.matmul(u_ps[:], wu[:, j, dc, :], xf[:, dc, :],
                                         start=(dc == 0), stop=(dc == DC - 1))
                    nc.scalar.activation(g[:, ft, :], u_ps[:], AF.Gelu,
                                         bias=bu[:, ft:ft + 1], scale=1.0)

            # ---- FFN down + bias; bias-add split Vector/Scalar and the
            # output DMA split across two queues to shorten the tail ----
            TH = T // 2
            for dt in range(DT):
                o_ps = psum.tile([P, T], F32, tag="ps", name=f"o{dt}")
                for h in range(2):
                    wd = wd_tiles.pop((dt, h))
                    for fc in range(FT // 2):
                        fcg = h * (FT // 2) + fc
                        nc.tensor.matmul(o_ps[:], wd[:, fc, :], g[:, fcg, :],
                                         start=(fcg == 0), stop=(fcg == FT - 1))
                    if dt + 2 < DT:
                        wdt = wdstream.tile([P, FT // 2, P], BF16, tag="wd",
                                            name=f"wd{dt + 2}_{h}")
                        nc.sync.dma_start(wdt[:], wd_d[(dt + 2) * 2 + h])
                        wd_tiles[(dt + 2, h)] = wdt
                otv = otp.tile([P, TH], BF16, tag="otv", name=f"otv{dt}")
                nc.vector.tensor_scalar_add(otv[:], o_ps[:, 0:TH],
                                            bd[:, dt:dt + 1])
                nc.sync.dma_start(out_d[dt, 0], otv[:])
                otg = otp.tile([P, TH], BF16, tag="otg", name=f"otg{dt}")
                nc.scalar.activation(otg[:], o_ps[:, TH:T], AF.Identity,
                                     bias=bd[:, dt:dt + 1], scale=1.0)
                nc.scalar.dma_start(out_d[dt, 1], otg[:])

    nc.finalize()
    return nc


def _get_nc():
    if _BUILT[0] is None:
        _BUILT[0] = _build_nc()
    return _BUILT[0]


def kernel(x, neuron_idx, neuron_weights, neuron_recipe, basis_A,
           w_up_w, w_up_b, w_down_w, w_down_b, alpha):
    import ml_dtypes
    nc = _get_nc()

    x = np.asarray(x, dtype=np.float32).reshape(NCORES * T, D)
    idxf = np.asarray(neuron_idx).astype(np.float32).reshape(NCORES * T, K)
    wgt = np.asarray(neuron_weights, dtype=np.float32).reshape(NCORES * T, K)
    rec = np.asarray(neuron_recipe, dtype=np.float32)
    bA = np.asarray(basis_A, dtype=np.float32)
    wu = np.asarray(w_up_w, dtype=np.float32)
    bu_in = np.asarray(w_up_b, dtype=np.float32)
    wd = np.asarray(w_down_w, dtype=np.float32)
    bd_in = np.asarray(w_down_b, dtype=np.float32)
    alpha_f = float(np.asarray(alpha, dtype=np.float32))

    # replicated operands, packed into the on-device layouts
    # A1 in fp8, scaled by 16 to stay in e4m3 normal range (1/16 in C)
    a1 = np.ascontiguousarray(
        np.clip(bA.transpose(1, 0, 2).reshape(D, NB * R) * A1S, -240.0, 240.0)
        .reshape(DC, P, NB * R).transpose(1, 0, 2)
    ).astype(ml_dtypes.float8_e4m3)
    # delta projection in fp8: 32*alpha*A2, [(pair, j, p), (dt, c)] packed
    a2m = np.clip(bA.transpose(0, 2, 1).reshape(NB * R, D) * (alpha_f * A2S),
                  -240.0, 240.0)
    a2 = np.ascontiguousarray(
        a2m.reshape(2, 2, P, DT, P).transpose(2, 0, 1, 3, 4)
    ).astype(ml_dtypes.float8_e4m3)
    wu_p = np.ascontiguousarray(
        wu.reshape(DC, P, FT // 2, 2, P).transpose(2, 1, 3, 0, 4)
    ).astype(ml_dtypes.bfloat16)
    wd_p = np.ascontiguousarray(
        wd.reshape(2, FT // 2, P, DT, P).transpose(3, 0, 2, 1, 4)
        .reshape(DT * 2, P, FT // 2, P)).astype(ml_dtypes.bfloat16)

    # packed per-core blob: idxw + C + identities + biases + recipe, bf16
    blobc = np.zeros((P, BL_W), dtype=np.float32)
    # C = qred @ trep fused: C[q, p] = 1/A1S iff q % R == p % R
    blobc[:, BL_C:BL_C + P] = (
        np.arange(P)[:, None] % R == np.arange(P)[None, :] % R) / A1S
    blobc[:, BL_ID:BL_ID + P] = np.eye(P, dtype=np.float32)
    blobc[:, BL_ID32:BL_ID32 + P] = np.eye(P, dtype=np.float32) * A2S
    blobc[:, BL_BU:BL_BU + FT] = bu_in.reshape(FT, P).T
    blobc[:, BL_BD:BL_BD + DT] = bd_in.reshape(DT, P).T
    blobc[:NN, BL_REC:BL_REC + NB] = rec

    # SEL[n, i, m] = 1 iff n in [4i, 4i+4) and m // 32 == n - 4i
    sel = np.zeros((NB, NRT, P), dtype=np.float32)
    for n in range(NB):
        i, nloc = divmod(n, NRT)
        sel[n, i, nloc * R:(nloc + 1) * R] = 1.0
    sel = sel.astype(ml_dtypes.bfloat16)

    shared = {
        "sel": sel, "a1": a1, "a2": a2, "wu": wu_p, "wd": wd_p,
    }
    in_maps = []
    idxw = np.concatenate([idxf, wgt], axis=1).astype(
        ml_dtypes.bfloat16)  # [N*T, 16]
    for c in range(NCORES):
        xc = x[c * T:(c + 1) * T]  # [T, D]
        xtc = np.ascontiguousarray(xc.T.reshape(DC, P, T).transpose(1, 0, 2))
        xtbc = xtc.astype(ml_dtypes.bfloat16)
        xqs = {f"xq{q}": np.ascontiguousarray(xtbc[:, 2 * q:2 * q + 2, :])
               for q in range(4)}
        iwc = np.ascontiguousarray(
            idxw[c * T:(c + 1) * T].reshape(TT, P, 2 * K).transpose(1, 0, 2))
        blob_c = blobc.astype(ml_dtypes.bfloat16)
        blob_c[:, BL_IDX:BL_IDX + TT * 2 * K] = iwc.reshape(P, TT * 2 * K)
        in_maps.append({**xqs, "blob1": blob_c, **shared})

    res = run_bass_kernel_spmd(nc, in_maps, core_ids=list(range(NCORES)))

    out = np.empty((NCORES * T, D), dtype=np.float32)
    for c in range(NCORES):
        oq = res.results[c]["outT"].astype(np.float32)  # [DT, 2, P, TH]
        ot = oq.transpose(2, 0, 1, 3).reshape(P, DT, T)
        out[c * T:(c + 1) * T] = ot.transpose(1, 0, 2).reshape(D, T).T
    return out.reshape(2, 2048, D)
